# revision 17
# baseline (speedup 1.0000x reference)
"""Causal multi-head attention (B=8, H=16, S=1024, D=64, fp32) on 8 TRN2 cores.

Sharding: the B*H = 128 independent attention instances are split 16 per
core (pure data parallel, no collectives).

v2 design — heads processed in PAIRS (A, B) to fill the 128-wide machine:

  - Q, K, V loaded in natural s-order ([p = s%128, c = s//128]) with the
    fp32->fp16 cast done by the (gpsimd-initiated) DMA itself.
  - Q^T/K^T built by fp16 PE transposes of [128, 64] slices; head A lands
    on PSUM partitions 0-63, head B on 64-127 (col-tiled, so the two
    transposes run concurrently in the array). One full-width DVE copy
    per tensor then yields qt/kt [128, S] fp16 holding both heads.
  - Scores transposed: S^T[k, q] per k-tile, computed as two row-tiled
    concurrent matmuls (head A on array rows 0-63, head B on 64-127),
    written to adjacent PSUM banks so ONE exp activation (scale=1/8)
    covers both heads — halving ACT instruction overhead. No
    max-subtraction (scores are O(30), exp cannot overflow fp32/fp16);
    masked entries of the diagonal tile are zeroed with a 0/1 triangular
    mask on DVE, matching the reference's -10000 bias after softmax.
  - O[q, :] = P^T.T @ [V | 1] in fp16 (fp32 PSUM accumulation): the ones
    column accumulates the softmax denominator; a DVE reciprocal +
    per-partition tensor_scalar multiply normalizes into the staging
    tile, and one DMA per pair stores both heads.
"""

import numpy as np

import concourse.mybir as mybir
import concourse.tile as tile
from concourse import bacc
from concourse.bass_utils import run_bass_kernel_spmd
from concourse.masks import make_identity, make_upper_triangular

B, H, S, D = 8, 16, 1024, 64
NCORES = 8
HPC = B * H // NCORES  # heads per core
NPAIR = HPC // 2
P = 128
NQ = S // P
NK = S // P
F32 = mybir.dt.float32
F16 = mybir.dt.float16


def _score_chunks(w):
    """Split a width-w score row into PSUM-bank-sized pieces (<=512)."""
    out = []
    while w > 512:
        take = 512 if (w - 512 >= 256 or w == 1024) else w - 256
        out.append(take)
        w -= take
    out.append(w)
    return out


def _attention_body(ctx_pools, tc, out, q, k, v):
    nc = tc.nc

    const = ctx_pools.enter_context(tc.tile_pool(name="const", bufs=1))
    io = ctx_pools.enter_context(tc.tile_pool(name="io", bufs=2))
    vpool = ctx_pools.enter_context(tc.tile_pool(name="vpool", bufs=3))
    tp = ctx_pools.enter_context(tc.tile_pool(name="tp", bufs=2))
    ptp = ctx_pools.enter_context(tc.tile_pool(name="ptp", bufs=2))
    small = ctx_pools.enter_context(tc.tile_pool(name="small", bufs=4))
    obp = ctx_pools.enter_context(tc.tile_pool(name="obp", bufs=2))
    psum_t = ctx_pools.enter_context(tc.tile_pool(name="psum_t", bufs=2, space="PSUM"))
    psum_s = ctx_pools.enter_context(tc.tile_pool(name="psum_s", bufs=2, space="PSUM"))
    psum_o = ctx_pools.enter_context(tc.tile_pool(name="psum_o", bufs=2, space="PSUM"))

    ident = const.tile([P, P], F16)
    make_identity(nc, ident)
    # umask2[k, h, q] = 1.0 where q >= k, else 0 — one triangle per head slot.
    umask2 = const.tile([P, 2, P], F16)
    make_upper_triangular(nc, umask2[:, 0, :], val=1.0, diag=True)
    make_upper_triangular(nc, umask2[:, 1, :], val=1.0, diag=True)

    state = {}

    def stage_load(t):
        # Natural order: partition p holds rows s ≡ p (mod 128); the DMA
        # (gpsimd/SWDGE — the only caster) converts fp32->fp16 in flight.
        # q/k use [p, c, h, d] so one PE transpose of [128, (h d)] yields the
        # A/B-stacked Q^T/K^T rows directly, with contiguous 128-col weights.
        # One DMA per head: the 4-dim [p c h d] pattern can't balance.
        qn = io.tile([P, NK, 2, D], F16, tag="qn")
        kn = io.tile([P, NK, 2, D], F16, tag="kn")
        for src, dst in ((q, qn), (k, kn)):
            for h in (0, 1):
                nc.gpsimd.dma_start(
                    out=dst[:, :, h, :],
                    in_=src[2 * t + h].rearrange("(c p) d -> p c d", p=P),
                )
        vp = vpool.tile([P, 2, NK, D + 1], F16, tag="vp")
        nc.gpsimd.dma_start(
            out=vp[:, :, :, 0:D],
            in_=v[2 * t : 2 * t + 2].rearrange("h (c p) d -> p h c d", p=P),
        )
        nc.vector.memset(vp[:, :, :, D : D + 1], 1.0)
        state[t] = {"qn": qn, "kn": kn, "vp": vp}

    def stage_transpose(t):
        # qt/kt [128, S] fp16: rows 0-63 = head A's Q^T, rows 64-127 = head
        # B's. Per c-slice, the A and B transposes target different column
        # groups of the PE array and run concurrently; all 16 land in one
        # PSUM bank, moved to SBUF by a single full-width DVE copy.
        st_ = state[t]
        qt = tp.tile([P, S], F16, tag="qt")
        kt = tp.tile([P, S], F16, tag="kt")
        for src, dst in ((st_["qn"], qt), (st_["kn"], kt)):
            ps = psum_t.tile([P, NK, P], F16, tag="tps")
            for c in range(NK):
                # [128, (h d)] -> [128 (h d), 128]: both heads in one shot.
                nc.tensor.transpose(
                    ps[:, c, :], src[:, c, :, :].rearrange("p h d -> p (h d)"), ident
                )
            nc.vector.tensor_copy(
                out=dst.rearrange("p (c x) -> p c x", c=NK), in_=ps
            )
        st_["qt"], st_["kt"] = qt, kt

    def stage_scores(t):
        # P^T tiles per k-tile for both heads: pt[:, h, :]. Row-tiled
        # matmul pairs write adjacent PSUM banks; one exp covers both.
        st_ = state[t]
        qt, kt = st_["qt"], st_["kt"]
        pts = []
        for ki in range(NK):
            w_all = S - ki * P
            pt = ptp.tile([P, 2, w_all], F16, tag=f"pt{ki}")
            j0 = 0
            for w in _score_chunks(w_all):
                st = psum_s.tile([P, 2, 512], F32, tag="st")
                for h in (0, 1):
                    nc.tensor.matmul(
                        st[:, h, 0:w],
                        lhsT=kt[64 * h : 64 * h + 64, ki * P : (ki + 1) * P],
                        rhs=qt[64 * h : 64 * h + 64, ki * P + j0 : ki * P + j0 + w],
                        start=True,
                        stop=True,
                    )
                nc.scalar.activation(
                    out=pt[:, :, j0 : j0 + w],
                    in_=st[:, :, 0:w],
                    func=mybir.ActivationFunctionType.Exp,
                    scale=0.125,
                )
                j0 += w
            # Zero the below-diagonal entries of the diagonal block.
            nc.vector.tensor_mul(out=pt[:, :, 0:P], in0=pt[:, :, 0:P], in1=umask2)
            pts.append(pt)
        st_["pts"] = pts

    def stage_pv(t):
        # O[q-tile] = sum_ki P^T_ki.T @ [V_ki | 1] per head, then normalize
        # into the pair staging tile; one DMA stores both heads.
        st_ = state.pop(t)
        pts, vp = st_["pts"], st_["vp"]
        oh = obp.tile([P, 2, NQ, D], F32, tag="oh")
        for qi in range(NQ):
            ot = psum_o.tile([P, 2, 66], F32, tag="ot")
            for ki in range(qi + 1):
                for h in (0, 1):
                    # start=True marks the whole 2KB bank row pending-zero,
                    # so only the first matmul touching the bank may set it;
                    # head B's first matmul overwrites via those bits.
                    nc.tensor.matmul(
                        ot[:, h, 0 : D + 1],
                        lhsT=pts[ki][:, h, (qi - ki) * P : (qi - ki + 1) * P],
                        rhs=vp[:, h, ki, :],
                        start=(ki == 0 and h == 0),
                        stop=(ki == qi),
                    )
            rec = small.tile([P, 2, 1], F32, tag="rec")
            nc.vector.reciprocal(rec, ot[:, :, D : D + 1])
            for h in (0, 1):
                nc.vector.tensor_scalar_mul(
                    oh[:, h, qi, :], ot[:, h, 0:D], rec[:, h, :]
                )
        nc.sync.dma_start(
            out=out[2 * t : 2 * t + 2].rearrange("h (c p) d -> p h c d", p=P),
            in_=oh,
        )

    # Software-pipelined emission: each engine's serial instruction stream
    # gets work whose inputs were produced a full stage earlier.
    stages = (stage_load, stage_transpose, stage_scores, stage_pv)
    for step in range(NPAIR + len(stages) - 1):
        for si in range(len(stages) - 1, -1, -1):
            tt = step - si
            if 0 <= tt < NPAIR:
                stages[si](tt)


_NC_CACHE = {}


def _build(nrep=1):
    if nrep in _NC_CACHE:
        return _NC_CACHE[nrep]
    from contextlib import ExitStack

    nc = bacc.Bacc(trn_type="TRN2", target_bir_lowering=False, debug=False)
    q = nc.dram_tensor("q", [HPC, S, D], F32, kind="ExternalInput").ap()
    k = nc.dram_tensor("k", [HPC, S, D], F32, kind="ExternalInput").ap()
    v = nc.dram_tensor("v", [HPC, S, D], F32, kind="ExternalInput").ap()
    out = nc.dram_tensor("out", [HPC, S, D], F32, kind="ExternalOutput").ap()
    with tile.TileContext(nc) as tc:
        for _ in range(nrep):
            with ExitStack() as pools:
                _attention_body(pools, tc, out, q, k, v)
    nc.compile()
    _NC_CACHE[nrep] = nc
    return nc


def run(inputs, trace=False):
    """Run on 8 cores; returns (full_output, exec_time_ns_or_None)."""
    nc = _build()
    q = np.ascontiguousarray(np.asarray(inputs["q"], dtype=np.float32)).reshape(
        B * H, S, D
    )
    k = np.ascontiguousarray(np.asarray(inputs["k"], dtype=np.float32)).reshape(
        B * H, S, D
    )
    v = np.ascontiguousarray(np.asarray(inputs["v"], dtype=np.float32)).reshape(
        B * H, S, D
    )
    in_maps = [
        {
            "q": q[i * HPC : (i + 1) * HPC],
            "k": k[i * HPC : (i + 1) * HPC],
            "v": v[i * HPC : (i + 1) * HPC],
        }
        for i in range(NCORES)
    ]
    res = run_bass_kernel_spmd(nc, in_maps, list(range(NCORES)), trace=trace)
    full = np.concatenate([res.results[i]["out"] for i in range(NCORES)], axis=0)
    return full.reshape(B, H, S, D), res.exec_time_ns


def kernel(q, k, v):
    out, _ = run({"q": q, "k": k, "v": v})
    return out


# revision 22
# speedup vs baseline: 1.4291x; 1.4291x over previous
"""Causal multi-head attention (B=8, H=16, S=1024, D=64, fp32) on 8 TRN2 cores.

Sharding: the B*H = 128 independent attention instances are split 16 per
core (pure data parallel, no collectives).

v2 design — heads processed in PAIRS (A, B) to fill the 128-wide machine:

  - Q, K, V loaded in natural s-order ([p = s%128, c = s//128]) with the
    fp32->fp16 cast done by the (gpsimd-initiated) DMA itself.
  - Q^T/K^T built by fp16 PE transposes of [128, 64] slices; head A lands
    on PSUM partitions 0-63, head B on 64-127 (col-tiled, so the two
    transposes run concurrently in the array). One full-width DVE copy
    per tensor then yields qt/kt [128, S] fp16 holding both heads.
  - Scores transposed: S^T[k, q] per k-tile, computed as two row-tiled
    concurrent matmuls (head A on array rows 0-63, head B on 64-127),
    written to adjacent PSUM banks so ONE exp activation (scale=1/8)
    covers both heads — halving ACT instruction overhead. No
    max-subtraction (scores are O(30), exp cannot overflow fp32/fp16);
    masked entries of the diagonal tile are zeroed with a 0/1 triangular
    mask on DVE, matching the reference's -10000 bias after softmax.
  - O[q, :] = P^T.T @ [V | 1] in fp16 (fp32 PSUM accumulation): the ones
    column accumulates the softmax denominator; a DVE reciprocal +
    per-partition tensor_scalar multiply normalizes into the staging
    tile, and one DMA per pair stores both heads.
"""

import numpy as np

import concourse.mybir as mybir
import concourse.tile as tile
from concourse import bacc
from concourse.bass_utils import run_bass_kernel_spmd
from concourse.masks import make_identity, make_upper_triangular

B, H, S, D = 8, 16, 1024, 64
NCORES = 8
HPC = B * H // NCORES  # heads per core
NPAIR = HPC // 2
P = 128
NQ = S // P
NK = S // P
F32 = mybir.dt.float32
F16 = mybir.dt.float16


def _score_chunks(w):
    """Split a width-w score row into PSUM-bank-sized pieces (<=512)."""
    out = []
    while w > 512:
        take = 512 if (w - 512 >= 256 or w == 1024) else w - 256
        out.append(take)
        w -= take
    out.append(w)
    return out


def _attention_body(ctx_pools, tc, out, q, k, v):
    nc = tc.nc

    const = ctx_pools.enter_context(tc.tile_pool(name="const", bufs=1))
    io = ctx_pools.enter_context(tc.tile_pool(name="io", bufs=2))
    vpool = ctx_pools.enter_context(tc.tile_pool(name="vpool", bufs=3))
    tp = ctx_pools.enter_context(tc.tile_pool(name="tp", bufs=2))
    ptp = ctx_pools.enter_context(tc.tile_pool(name="ptp", bufs=2))
    small = ctx_pools.enter_context(tc.tile_pool(name="small", bufs=4))
    obp = ctx_pools.enter_context(tc.tile_pool(name="obp", bufs=2))
    psum_t = ctx_pools.enter_context(tc.tile_pool(name="psum_t", bufs=2, space="PSUM"))
    psum_s = ctx_pools.enter_context(tc.tile_pool(name="psum_s", bufs=2, space="PSUM"))
    psum_o = ctx_pools.enter_context(tc.tile_pool(name="psum_o", bufs=2, space="PSUM"))

    ident = const.tile([P, P], F16)
    make_identity(nc, ident)
    # umask2[k, h, q] = 1.0 where q >= k, else 0 — one triangle per head slot.
    umask2 = const.tile([P, 2, P], F16)
    make_upper_triangular(nc, umask2[:, 0, :], val=1.0, diag=True)
    make_upper_triangular(nc, umask2[:, 1, :], val=1.0, diag=True)

    state = {}

    def stage_load(t):
        # Natural order: partition p holds rows s ≡ p (mod 128); the DMA
        # (gpsimd/SWDGE — the only caster) converts fp32->fp16 in flight.
        # q/k use [p, c, h, d] so one PE transpose of [128, (h d)] yields the
        # A/B-stacked Q^T/K^T rows directly, with contiguous 128-col weights.
        # One DMA per head: the 4-dim [p c h d] pattern can't balance.
        qn = io.tile([P, NK, 2, D], F16, tag="qn")
        kn = io.tile([P, NK, 2, D], F16, tag="kn")
        for src, dst in ((q, qn), (k, kn)):
            for h in (0, 1):
                nc.gpsimd.dma_start(
                    out=dst[:, :, h, :],
                    in_=src[2 * t + h].rearrange("(c p) d -> p c d", p=P),
                )
        vp = vpool.tile([P, 2, NK, D + 1], F16, tag="vp")
        nc.gpsimd.dma_start(
            out=vp[:, :, :, 0:D],
            in_=v[2 * t : 2 * t + 2].rearrange("h (c p) d -> p h c d", p=P),
        )
        nc.vector.memset(vp[:, :, :, D : D + 1], 1.0)
        state[t] = {"qn": qn, "kn": kn, "vp": vp}

    def stage_transpose(t):
        # qt/kt [128, S] fp16: rows 0-63 = head A's Q^T, rows 64-127 = head
        # B's. Per c-slice, the A and B transposes target different column
        # groups of the PE array and run concurrently; all 16 land in one
        # PSUM bank, moved to SBUF by a single full-width DVE copy.
        st_ = state[t]
        qt = tp.tile([P, S], F16, tag="qt")
        kt = tp.tile([P, S], F16, tag="kt")
        for src, dst in ((st_["qn"], qt), (st_["kn"], kt)):
            ps = psum_t.tile([P, NK, P], F16, tag="tps")
            for c in range(NK):
                # [128, (h d)] -> [128 (h d), 128]: both heads in one shot.
                nc.tensor.transpose(
                    ps[:, c, :], src[:, c, :, :].rearrange("p h d -> p (h d)"), ident
                )
            nc.vector.tensor_copy(
                out=dst.rearrange("p (c x) -> p c x", c=NK), in_=ps
            )
        st_["qt"], st_["kt"] = qt, kt

    def stage_scores(t):
        # P^T tiles per k-tile for both heads: pt[:, h, :]. Row-tiled
        # matmul pairs write adjacent PSUM banks; one exp covers both.
        st_ = state[t]
        qt, kt = st_["qt"], st_["kt"]
        pts = []
        for ki in range(NK):
            w_all = S - ki * P
            pt = ptp.tile([P, 2, w_all], F16, tag=f"pt{ki}")
            j0 = 0
            for w in _score_chunks(w_all):
                st = psum_s.tile([P, 2, 512], F32, tag="st")
                for h in (0, 1):
                    nc.tensor.matmul(
                        st[:, h, 0:w],
                        lhsT=kt[64 * h : 64 * h + 64, ki * P : (ki + 1) * P],
                        rhs=qt[64 * h : 64 * h + 64, ki * P + j0 : ki * P + j0 + w],
                        start=True,
                        stop=True,
                    )
                nc.scalar.activation(
                    out=pt[:, :, j0 : j0 + w],
                    in_=st[:, :, 0:w],
                    func=mybir.ActivationFunctionType.Exp,
                    scale=0.125,
                )
                j0 += w
            # Zero the below-diagonal entries of the diagonal block.
            nc.vector.tensor_mul(out=pt[:, :, 0:P], in0=pt[:, :, 0:P], in1=umask2)
            pts.append(pt)
        st_["pts"] = pts

    def stage_pv(t):
        # O[q-tile] = sum_ki P^T_ki.T @ [V_ki | 1] per head, then normalize
        # into the pair staging tile; one DMA stores both heads.
        st_ = state.pop(t)
        pts, vp = st_["pts"], st_["vp"]
        oh = obp.tile([P, 2, NQ, D], F32, tag="oh")
        for qi in range(NQ):
            ot = psum_o.tile([P, 2, 66], F32, tag="ot")
            for ki in range(qi + 1):
                for h in (0, 1):
                    # start=True marks the whole 2KB bank row pending-zero,
                    # so only the first matmul touching the bank may set it;
                    # head B's first matmul overwrites via those bits.
                    nc.tensor.matmul(
                        ot[:, h, 0 : D + 1],
                        lhsT=pts[ki][:, h, (qi - ki) * P : (qi - ki + 1) * P],
                        rhs=vp[:, h, ki, :],
                        start=(ki == 0 and h == 0),
                        stop=(ki == qi),
                    )
            rec = small.tile([P, 2, 1], F32, tag="rec")
            nc.vector.reciprocal(rec, ot[:, :, D : D + 1])
            for h in (0, 1):
                nc.vector.tensor_scalar_mul(
                    oh[:, h, qi, :], ot[:, h, 0:D], rec[:, h, :]
                )
        nc.sync.dma_start(
            out=out[2 * t : 2 * t + 2].rearrange("h (c p) d -> p h c d", p=P),
            in_=oh,
        )

    # Software-pipelined emission: each engine's serial instruction stream
    # gets work whose inputs were produced a full stage earlier.
    stages = (stage_load, stage_transpose, stage_scores, stage_pv)
    for step in range(NPAIR + len(stages) - 1):
        for si in range(len(stages) - 1, -1, -1):
            tt = step - si
            if 0 <= tt < NPAIR:
                stages[si](tt)


_NC_CACHE = {}


def _build(nrep=1):
    if nrep in _NC_CACHE:
        return _NC_CACHE[nrep]
    from contextlib import ExitStack

    nc = bacc.Bacc(trn_type="TRN2", target_bir_lowering=False, debug=False)
    q = nc.dram_tensor("q", [HPC, S, D], F32, kind="ExternalInput").ap()
    k = nc.dram_tensor("k", [HPC, S, D], F32, kind="ExternalInput").ap()
    v = nc.dram_tensor("v", [HPC, S, D], F32, kind="ExternalInput").ap()
    out = nc.dram_tensor("out", [HPC, S, D], F32, kind="ExternalOutput").ap()
    with tile.TileContext(nc) as tc:
        for _ in range(nrep):
            with ExitStack() as pools:
                _attention_body(pools, tc, out, q, k, v)
    nc.compile()
    _NC_CACHE[nrep] = nc
    return nc


def run(inputs, trace=False):
    """Run on 8 cores; returns (full_output, exec_time_ns_or_None)."""
    nc = _build()
    q = np.ascontiguousarray(np.asarray(inputs["q"], dtype=np.float32)).reshape(
        B * H, S, D
    )
    k = np.ascontiguousarray(np.asarray(inputs["k"], dtype=np.float32)).reshape(
        B * H, S, D
    )
    v = np.ascontiguousarray(np.asarray(inputs["v"], dtype=np.float32)).reshape(
        B * H, S, D
    )
    in_maps = [
        {
            "q": q[i * HPC : (i + 1) * HPC],
            "k": k[i * HPC : (i + 1) * HPC],
            "v": v[i * HPC : (i + 1) * HPC],
        }
        for i in range(NCORES)
    ]
    res = run_bass_kernel_spmd(nc, in_maps, list(range(NCORES)), trace=trace)
    full = np.concatenate([res.results[i]["out"] for i in range(NCORES)], axis=0)
    return full.reshape(B, H, S, D), res.exec_time_ns


def kernel(q, k, v):
    out, _ = run({"q": q, "k": k, "v": v})
    return out
